# revision 34
# baseline (speedup 1.0000x reference)
"""Trainium2 Bass kernel for nn_DNFLayer (fuzzy DNF layer).

Strategy
--------
Data-parallel over batch B=32 across 8 cores (4 batches/core). Per core the
(i, j) permutation grid is padded to the full 32x32 grid (diagonal masked via
the OR-kernel broadcast), giving 4096 rows = 32 row-tiles of 128 partitions.

The conjunct product over the 112 inputs is factorized per permutation
(i, j):  conj = F0(b) * FU1(b,i) * FU2(b,j) * FB1(b,i,j) * FB2(b,j,i),
each factor being a product of per-channel affine terms (alpha*x + beta)
evaluated in the gamma form  prod(alpha x + beta) = prod(beta) * prod(gamma x
+ 1), gamma = alpha/beta.  All weight-only quantities (softmax(and_kernel)
-> gamma, the per-(r,d) beta products, sigmoid(or_kernel), diagonal mask)
are precomputed on the host in fp64 and DMAed in pre-broadcast, so the
device spends zero time on the weight path and the binary eval starts as
soon as its DMAs land.

Engine split: the heavy bf16 eval + product trees run on the vector engine
(2x mode); the +1 bias passes run on the scalar engine; gpsimd is used for
bulk/late input DMAs. The final per-object reductions use one PE transpose
of the [128, 96] disjunct-complement tile.
"""

import numpy as np
import ml_dtypes

BF = ml_dtypes.bfloat16
B, N, P0, P1, P2, R, D = 32, 32, 16, 32, 16, 3, 8
RD = R * D              # 24
NCORE = 8
BL = B // NCORE         # 4 batches per core
NT = BL * 8             # 32 row-tiles of 128 per core

_CACHE = {}


def _build():
    import concourse.tile as tile
    from concourse import mybir, bacc

    F32 = mybir.dt.float32
    B16 = mybir.dt.bfloat16
    MUL = mybir.AluOpType.mult
    ADD = mybir.AluOpType.add
    AF = mybir.ActivationFunctionType

    nc = bacc.Bacc("TRN2", target_bir_lowering=False, debug=False,
                   num_devices=NCORE)

    # ---- parameters (per-core shards / replicated constants) ----
    x_all_in = nc.declare_dram_parameter("x_all", [128, NT * 32], B16, isOutput=False)
    xu_in = nc.declare_dram_parameter("xu", [128, 80], B16, isOutput=False)
    gB_in = nc.declare_dram_parameter("gB", [128, 768], B16, isOutput=False)
    gU_in = nc.declare_dram_parameter("gU", [128, 1536], B16, isOutput=False)
    gN_in = nc.declare_dram_parameter("gN", [128, 384], B16, isOutput=False)
    okm_in = nc.declare_dram_parameter("okm", [128, 192], B16, isOutput=False)
    sel_in = nc.declare_dram_parameter("selcat", [32, 1152], B16, isOutput=False)
    ident_in = nc.declare_dram_parameter("ident", [128, 128], F32, isOutput=False)
    oldb_in = nc.declare_dram_parameter("olds_bin", [128, NT], F32, isOutput=False)
    oldu_in = nc.declare_dram_parameter("olds_un", [33, 4], F32, isOutput=False)

    out_binm = nc.declare_dram_parameter("out_binm", [128, NT], F32, isOutput=True)
    out_unm = nc.declare_dram_parameter("out_unm", [33, 4], F32, isOutput=True)

    with tile.TileContext(nc) as tc:
        with tc.tile_pool(name="cb", bufs=1) as cb, \
             tc.tile_pool(name="wk", bufs=1) as wk, \
             tc.tile_pool(name="ps", bufs=1, space="PSUM") as ps:

            # ---------- t0: input DMAs ----------
            # sync queue carries only the latency-critical chain (each
            # dma_start costs ~600ns of SP sequencer time, serialized);
            # bulk/late tensors ride the gpsimd DGE.
            gB = cb.tile([128, 768], B16)
            nc.sync.dma_start(gB[:], gB_in[:])
            x_all = cb.tile([128, NT * 32], B16)
            for h in range(4):
                nc.sync.dma_start(x_all[:, h * 256:(h + 1) * 256],
                                  x_all_in[:, h * 256:(h + 1) * 256])
            xu = cb.tile([128, 80], B16)
            nc.gpsimd.dma_start(xu[:], xu_in[:])
            gUs = cb.tile([128, 1536], B16)
            nc.gpsimd.dma_start(gUs[:], gU_in[:])
            gNs = cb.tile([128, 384], B16)
            nc.gpsimd.dma_start(gNs[:], gN_in[:])
            okmB = cb.tile([128, 192], B16)
            nc.gpsimd.dma_start(okmB[:], okm_in[:])
            sel = cb.tile([32, 1152], B16)
            nc.gpsimd.dma_start(sel[:], sel_in[:])
            ident = cb.tile([128, 128], F32)
            nc.gpsimd.dma_start(ident[:], ident_in[:])
            oldb = cb.tile([128, NT], F32)
            nc.gpsimd.dma_start(oldb[:], oldb_in[:])
            oldu = cb.tile([33, 4], F32)
            nc.gpsimd.dma_start(oldu[:], oldu_in[:])

            # ---------- phase C: unary/nullary factor pass ----
            emU = wk.tile([128, 1536], B16)

            def _emu(dst_lo, n_grp, x_lo, src_lo):
                nc.vector.tensor_tensor(
                    emU[:, dst_lo:dst_lo + n_grp * 32]
                        .rearrange("p (g c) -> p g c", c=32),
                    xu[:, x_lo:x_lo + 32].unsqueeze(1)
                        .broadcast_to((128, n_grp, 32)),
                    gUs[:, src_lo:src_lo + n_grp * 32]
                        .rearrange("p (g c) -> p g c", c=32), op=MUL)
            _emu(0, 16, 0, 0)
            _emu(512, 8, 0, 512)
            _emu(768, 8, 32, 768)
            _emu(1024, 16, 32, 1024)
            nc.scalar.activation(emU[:], emU[:], AF.Copy, bias=1.0)
            # U tree: [128, 48, 32] -> [128, 48]
            cur = emU[:].rearrange("p (g c) -> p g c", c=32)
            for w in (16, 8, 4, 2):
                nxt = wk.tile([128, 48 * w], B16, tag=f"ut{w}")
                nc.vector.tensor_tensor(
                    nxt[:].rearrange("p (g c) -> p g c", c=w),
                    cur[:, :, 0:w], cur[:, :, w:2 * w], op=MUL)
                cur = nxt[:].rearrange("p (g c) -> p g c", c=w)
            fu12 = wk.tile([128, 48], B16)
            nc.vector.tensor_tensor(fu12[:].unsqueeze(2), cur[:, :, 0:1],
                                    cur[:, :, 1:2], op=MUL)

            emN = wk.tile([128, 384], B16)
            nc.vector.tensor_tensor(
                emN[:].rearrange("p (r c) -> p r c", r=24),
                xu[:, 64:80].unsqueeze(1).broadcast_to((128, 24, 16)),
                gNs[:].rearrange("p (r c) -> p r c", r=24), op=MUL)
            nc.scalar.activation(emN[:], emN[:], AF.Copy, bias=1.0)
            cur = emN[:].rearrange("p (g c) -> p g c", c=16)
            for w in (8, 4, 2):
                nxt = wk.tile([128, 24 * w], B16, tag=f"nt{w}")
                nc.vector.tensor_tensor(
                    nxt[:].rearrange("p (g c) -> p g c", c=w),
                    cur[:, :, 0:w], cur[:, :, w:2 * w], op=MUL)
                cur = nxt[:].rearrange("p (g c) -> p g c", c=w)
            f0g = wk.tile([128, 24], B16)
            nc.vector.tensor_tensor(f0g[:].unsqueeze(2), cur[:, :, 0:1],
                                    cur[:, :, 1:2], op=MUL)

            fu2f0 = wk.tile([128, 24], B16)
            nc.vector.tensor_tensor(fu2f0[:], fu12[:, 24:48], f0g[:], op=MUL)

            # ---------- phase D: per-b row broadcasts via PE ----------
            FU1B = cb.tile([128, 768], B16)
            FU2F0B = cb.tile([128, 96], B16)
            for b in range(BL):
                rhs1 = wk.tile([32, 24], B16, tag="rhs1")
                nc.vector.tensor_copy(rhs1[:], fu12[b * 32:(b + 1) * 32, 0:24])
                rhs2 = wk.tile([32, 24], B16, tag="rhs2")
                nc.vector.tensor_copy(rhs2[:], fu2f0[b * 32:(b + 1) * 32, :])
                psF = ps.tile([128, 192], F32, tag="pp")
                for t in range(8):
                    nc.tensor.matmul(psF[:, t * 24:(t + 1) * 24],
                                     sel[0:32, t * 128:(t + 1) * 128],
                                     rhs1[:], start=True, stop=True)
                nc.vector.tensor_copy(FU1B[:, b * 192:(b + 1) * 192], psF[:])
                psJ = ps.tile([128, 24], F32, tag="pj")
                nc.tensor.matmul(psJ[:], sel[0:32, 1024:1152],
                                 rhs2[:], start=True, stop=True)
                nc.vector.tensor_copy(FU2F0B[:, b * 24:(b + 1) * 24], psJ[:])

            # PFOK[p, (b,t,rd)] = FU1B * FU2F0B(bcast t) * okmB(bcast b)
            PFOK = cb.tile([128, 768], B16)
            nc.vector.tensor_tensor(
                PFOK[:].rearrange("p (b t r) -> p b t r", b=4, t=8),
                FU1B[:].rearrange("p (b t r) -> p b t r", b=4, t=8),
                FU2F0B[:].rearrange("p (b r) -> p b r", b=4)
                    .unsqueeze(2).broadcast_to((128, 4, 8, 24)), op=MUL)
            nc.vector.tensor_tensor(
                PFOK[:].rearrange("p (b t r) -> p b t r", b=4, t=8),
                PFOK[:].rearrange("p (b t r) -> p b t r", b=4, t=8),
                okmB[:].rearrange("p (t r) -> p t r", t=8)
                    .unsqueeze(1).broadcast_to((128, 4, 8, 24)), op=MUL)

            # ---------- phase E: main binary pipeline ----------
            em = wk.tile([128, NT * 768], B16)
            t1 = wk.tile([128, NT * 384], B16)
            t2 = wk.tile([128, NT * 192], B16)
            t3 = wk.tile([128, NT * 96], B16)
            t4 = wk.tile([128, NT * 48], B16)
            cj = wk.tile([128, NT * 24], B16)
            gA = wk.tile([128, 768], B16)
            d1 = wk.tile([128, 384], B16)
            d2 = wk.tile([128, 192], B16)
            pdF = wk.tile([128, 96], F32)

            # evals (vector 2x) + bias (+1): b0-b2 scalar, b3 vector 4x
            for b in range(BL):
                nc.vector.tensor_tensor(
                    em[:, b * 6144:(b + 1) * 6144]
                        .rearrange("p (k r c) -> p k r c", k=8, r=24),
                    x_all[:, b * 256:(b + 1) * 256]
                        .rearrange("p (k c) -> p k c", k=8)
                        .unsqueeze(2).broadcast_to((128, 8, 24, 32)),
                    gB[:].rearrange("p (r c) -> p r c", r=24)
                        .unsqueeze(1).broadcast_to((128, 8, 24, 32)), op=MUL)
                if b == BL - 1:
                    # bias only the upper channel halves; L1 b3 applies the
                    # +1 to the lower halves inline via scalar_tensor_tensor
                    emv = em[:, b * 6144:(b + 1) * 6144].rearrange(
                        "p (g c) -> p g c", c=32)
                    nc.vector.tensor_scalar(emv[:, :, 16:32],
                                            emv[:, :, 16:32],
                                            1.0, None, op0=ADD)
                else:
                    nc.scalar.activation(em[:, b * 6144:(b + 1) * 6144],
                                         em[:, b * 6144:(b + 1) * 6144],
                                         AF.Copy, bias=1.0)

            # L1/L2 per b (pipelines behind each +1), L3..pd per b-pair
            for b in range(BL):
                cur = em[:, b * 6144:(b + 1) * 6144].rearrange(
                    "p (g c) -> p g c", c=32)
                for w, tl in ((16, t1), (8, t2)):
                    dst = tl[:, b * 192 * w:(b + 1) * 192 * w].rearrange(
                        "p (g c) -> p g c", c=w)
                    if b == BL - 1 and w == 16:
                        nc.vector.scalar_tensor_tensor(
                            dst, cur[:, :, 0:w], 1.0, cur[:, :, w:2 * w],
                            op0=ADD, op1=MUL)
                    else:
                        nc.vector.tensor_tensor(dst, cur[:, :, 0:w],
                                                cur[:, :, w:2 * w], op=MUL)
                    cur = dst
            for pb in range(2):
                cur = t2[:, pb * 3072:(pb + 1) * 3072].rearrange(
                    "p (g c) -> p g c", c=8)
                for w, tl in ((4, t3), (2, t4)):
                    dst = tl[:, pb * 384 * w:(pb + 1) * 384 * w].rearrange(
                        "p (g c) -> p g c", c=w)
                    nc.vector.tensor_tensor(dst, cur[:, :, 0:w],
                                            cur[:, :, w:2 * w], op=MUL)
                    cur = dst
                # disjunct chain per pair
                nc.vector.tensor_tensor(
                    cj[:, pb * 384:(pb + 1) * 384].unsqueeze(2),
                    cur[:, :, 0:1], cur[:, :, 1:2], op=MUL)
                cjb = cj[:, pb * 384:(pb + 1) * 384]
                nc.vector.tensor_tensor(cjb, cjb,
                                        PFOK[:, pb * 384:(pb + 1) * 384],
                                        op=MUL)
                gAb = gA[:, pb * 384:(pb + 1) * 384]
                nc.vector.tensor_scalar(gAb, cjb, -1.0, 1.0, op0=MUL, op1=ADD)
                d1b = d1[:, pb * 192:(pb + 1) * 192].rearrange(
                    "p (g dd) -> p g dd", dd=4)
                gvb = gAb.rearrange("p (g dd) -> p g dd", dd=8)
                nc.vector.tensor_tensor(d1b, gvb[:, :, 0:4], gvb[:, :, 4:8],
                                        op=MUL)
                d2b = d2[:, pb * 96:(pb + 1) * 96].rearrange(
                    "p (g dd) -> p g dd", dd=2)
                nc.vector.tensor_tensor(d2b, d1b[:, :, 0:2], d1b[:, :, 2:4],
                                        op=MUL)
                # write (r, k)-ordered pd: pdF[p, r*32 + b*8 + t]  (fp32 out)
                d2b4 = d2[:, pb * 96:(pb + 1) * 96].rearrange(
                    "p (b t r dd) -> p b t r dd", b=2, t=8, r=3)
                nc.vector.tensor_tensor(
                    pdF[:].rearrange("p (r k) -> p r k", r=3)
                        [:, :, pb * 16:(pb + 1) * 16]
                        .rearrange("p r (b t) -> p b t r", b=2).unsqueeze(4),
                    d2b4[:, :, :, :, 0:1], d2b4[:, :, :, :, 1:2], op=MUL)

                # binary merge per pair (r=2 block cols), early output DMA
                tb = wk.tile([128, 16], F32, tag=f"tb{pb}")
                ob = oldb[:, pb * 16:(pb + 1) * 16]
                nc.vector.tensor_scalar(tb[:], ob, -1.0, 1.0,
                                        op0=MUL, op1=ADD)
                nc.vector.tensor_tensor(
                    tb[:], tb[:], pdF[:, 64 + pb * 16:64 + (pb + 1) * 16],
                    op=MUL)
                nc.vector.tensor_scalar(tb[:], tb[:], -1.0, 1.0,
                                        op0=MUL, op1=ADD)
                nc.gpsimd.dma_start(
                    out_binm[:, pb * 16:(pb + 1) * 16], tb[:])

            # ---------- phase F: unary/nullary merges via PE transpose ----------
            pdT = ps.tile([96, 128], F32, tag="pt")
            nc.tensor.transpose(pdT[:], pdF[:], ident[:])
            pdS = wk.tile([64, 128], F32)
            nc.vector.tensor_copy(pdS[:], pdT[0:64, :])

            # shared product over j within i4-groups for rows 0:64 (r=0, r=1)
            cur = pdS[:].rearrange("p (i4 j) -> p i4 j", i4=4)
            for w in (16, 8, 4, 2, 1):
                nxt = wk.tile([64, 4 * w], F32, tag=f"pu{w}")
                nxtv = nxt[:].rearrange("p (i4 j) -> p i4 j", i4=4)
                nc.vector.tensor_tensor(nxtv, cur[:, :, 0:w], cur[:, :, w:2 * w],
                                        op=MUL)
                cur = nxtv
            j4 = nxt  # [64, 4]: rows 0:32 = r0 per (b,t,i4); rows 32:64 = r1
            pdu0 = wk.tile([32, 4], F32)
            nc.vector.tensor_copy(pdu0[:], j4[:][32:64, :])
            pdu = pdu0[:]
            tu = wk.tile([33, 4], F32)
            nc.vector.tensor_scalar(tu[0:32, :], oldu[0:32, :], -1.0, 1.0,
                                    op0=MUL, op1=ADD)
            nc.vector.tensor_tensor(tu[0:32, :], tu[0:32, :], pdu, op=MUL)

            # nullary: finish the product over i4 for rows 0:32 (r=0)
            cur = j4[:][0:32, :]
            for w in (2, 1):
                nxt = wk.tile([32, w], F32, tag=f"pn{w}")
                nc.vector.tensor_tensor(nxt[:], cur[:, 0:w], cur[:, w:2 * w],
                                        op=MUL)
                cur = nxt[:]
            # fold the remaining 32 partition values (b, t) -> per-b products
            q = wk.tile([32, 32], F32)
            nc.vector.memset(q[:], 1.0)
            nc.vector.tensor_copy(q[:, 0:1], cur)
            qT = wk.tile([32, 32], F32)
            nc.vector.transpose(qT[:], q[:])
            cur = qT[0:1, :].rearrange("p (b i8) -> p b i8", b=4)
            for w in (4, 2, 1):
                nxt = wk.tile([1, 4 * w], F32, tag=f"pq{w}")
                nxtv = nxt[:].rearrange("p (b i8) -> p b i8", b=4)
                nc.vector.tensor_tensor(nxtv, cur[:, :, 0:w], cur[:, :, w:2 * w],
                                        op=MUL)
                cur = nxtv
            pdn = cur.rearrange("p b i8 -> p (b i8)")  # [1, 4]
            on0 = wk.tile([1, 4], F32)
            nc.vector.tensor_copy(on0[:], oldu[32:33, :])
            tn = wk.tile([1, 4], F32)
            nc.vector.tensor_scalar(tn[:], on0[:], -1.0, 1.0, op0=MUL, op1=ADD)
            nc.vector.tensor_tensor(tn[:], tn[:], pdn, op=MUL)
            nc.vector.tensor_copy(tu[32:33, :], tn[:])
            nc.vector.tensor_scalar(tu[:], tu[:], -1.0, 1.0, op0=MUL, op1=ADD)
            nc.sync.dma_start(out_unm[:], tu[:])

    nc.compile()
    return nc


def _host_prep(nullary_preds, unary_preds, binary_preds, and_kernel, or_kernel):
    """Build per-core input maps (sharding + layout + weight-only prep)."""
    null_ = np.asarray(nullary_preds, np.float32)
    un = np.asarray(unary_preds, np.float32)
    bi = np.asarray(binary_preds, np.float32)
    ak = np.asarray(and_kernel, np.float64)
    ok = np.asarray(or_kernel, np.float64)

    I, J = np.meshgrid(np.arange(N), np.arange(N), indexing="ij")
    off = I != J
    Jm = J - (J > I)
    Im = I - (I > J)

    binP = np.zeros((B, N, N, P2), np.float32)
    binP[:, off] = bi[:, I[off], Jm[off]]
    binT = np.zeros((B, N, N, P2), np.float32)
    binT[:, off] = bi[:, J[off], Im[off]]
    binPT = np.concatenate([binP, binT], axis=-1)          # [B,32,32,32]

    # row-tile layout: x_all[core][p, k=(b,t), c] = binPT[4c+b, t*128+p, c]
    xg = binPT.reshape(NCORE, BL, 8, 128, 32)
    x_all = np.ascontiguousarray(xg.transpose(0, 3, 1, 2, 4)
                                 ).reshape(NCORE, 128, NT * 32).astype(BF)
    olds_bin = np.ascontiguousarray(
        binP[..., 15].reshape(NCORE, BL, 8, 128).transpose(0, 3, 1, 2)
    ).reshape(NCORE, 128, NT).astype(np.float32)

    # unary pass rows (b, i): [u | u | n]
    xun = np.concatenate(
        [un, un, np.broadcast_to(null_[:, None, :], (B, N, P0))], axis=-1)
    xu = xun.reshape(NCORE, 128, 80).astype(BF)
    # rows (b, i8), cols i4 : out_unm[q=(b*8+i//4), i%4]; row 32 = nullary
    olds_un = np.concatenate(
        [un[..., 31].reshape(NCORE, 4, 8, 4).reshape(NCORE, 32, 4),
         null_[:, 15].reshape(NCORE, 1, 4)], axis=1).astype(np.float32)

    # weight-only prep (fp64): softmax(and_kernel) -> gamma form + OR fold
    akT = np.ascontiguousarray(ak.transpose(2, 0, 1, 3)).reshape(112, RD, 3)
    e = np.exp(akT - akT.max(axis=-1, keepdims=True))      # [c, rd, 3]
    bsum = e[:, :, 1] + e[:, :, 2]
    stot = e.sum(axis=-1)
    gam = (e[:, :, 0] - e[:, :, 1]) / bsum                 # [c, rd]
    bprod = (bsum / stot).prod(axis=0)                     # [rd]
    sig = 1.0 / (1.0 + np.exp(-ok.reshape(RD)))            # [rd]
    okrow = sig * bprod                                    # [rd]

    gamT = gam.T                                           # [rd, c]
    gB = np.broadcast_to(gamT[:, 80:112].reshape(1, 768), (128, 768)).astype(BF)
    gU = np.broadcast_to(gamT[:, 16:80].reshape(1, 1536), (128, 1536)).astype(BF)
    gN = np.broadcast_to(gamT[:, 0:16].reshape(1, 384), (128, 384)).astype(BF)

    p = np.arange(128)
    t = np.arange(8)
    maskc = ((p[:, None] % 32) != (t[None, :] * 4 + p[:, None] // 32)
             ).astype(np.float64)
    okm = (maskc[:, :, None] * okrow[None, None, :]).reshape(128, 192).astype(BF)

    selT = (np.arange(32)[:, None, None] == (t[None, :, None] * 4 + p[None, None, :] // 32))
    selJ = (np.arange(32)[:, None] == (p[None, :] % 32))
    selcat = np.concatenate([selT.reshape(32, 1024), selJ], axis=1).astype(BF)
    ident = np.eye(128, dtype=np.float32)

    in_maps = []
    for c in range(NCORE):
        in_maps.append({
            "x_all": x_all[c],
            "xu": xu[c],
            "gB": gB,
            "gU": gU,
            "gN": gN,
            "okm": okm,
            "selcat": selcat,
            "ident": ident,
            "olds_bin": olds_bin[c],
            "olds_un": olds_un[c],
        })
    return in_maps


def _assemble(results, nullary_preds, unary_preds, binary_preds):
    null_ = np.asarray(nullary_preds, np.float32).copy()
    un = np.asarray(unary_preds, np.float32).copy()
    bi = np.asarray(binary_preds, np.float32).copy()

    I, J = np.meshgrid(np.arange(N), np.arange(N), indexing="ij")
    off = I != J
    Jm = J - (J > I)

    for c in range(NCORE):
        r = results[c]
        # out_binm [128, NT=(b,t)] -> rows[b, t*128+p]
        ob = r["out_binm"].reshape(128, BL, 8).transpose(1, 2, 0).reshape(BL, N, N)
        for bl in range(BL):
            b = c * BL + bl
            bi[b, I[off], Jm[off], 15] = ob[bl][off]
        un[c * BL:(c + 1) * BL, :, 31] = r["out_unm"][0:32].reshape(BL, 8, 4).reshape(BL, N)
        null_[c * BL:(c + 1) * BL, 15] = r["out_unm"][32].reshape(BL)

    return np.concatenate(
        [null_, un.reshape(B, -1), bi.reshape(B, -1)], axis=-1)


def kernel(nullary_preds, unary_preds, binary_preds, and_kernel, or_kernel):
    from concourse.bass_utils import run_bass_kernel_spmd

    if "nc" not in _CACHE:
        _CACHE["nc"] = _build()
    nc = _CACHE["nc"]

    in_maps = _host_prep(nullary_preds, unary_preds, binary_preds,
                         and_kernel, or_kernel)
    res = run_bass_kernel_spmd(nc, in_maps, list(range(NCORE)))
    return _assemble(res.results, nullary_preds, unary_preds, binary_preds)


if __name__ == "__main__":
    import reference as ref
    ins = {k: np.asarray(v) for k, v in ref.setup_inputs().items()}
    out = kernel(**ins)
    print("kernel out:", out.shape, out.dtype)


# revision 36
# speedup vs baseline: 1.0044x; 1.0044x over previous
"""Trainium2 Bass kernel for nn_DNFLayer (fuzzy DNF layer).

Strategy
--------
Data-parallel over batch B=32 across 8 cores (4 batches/core). Per core the
(i, j) permutation grid is padded to the full 32x32 grid (diagonal masked via
the OR-kernel broadcast), giving 4096 rows = 32 row-tiles of 128 partitions.

The conjunct product over the 112 inputs is factorized per permutation
(i, j):  conj = F0(b) * FU1(b,i) * FU2(b,j) * FB1(b,i,j) * FB2(b,j,i),
each factor being a product of per-channel affine terms (alpha*x + beta)
evaluated in the gamma form  prod(alpha x + beta) = prod(beta) * prod(gamma x
+ 1), gamma = alpha/beta.  All weight-only quantities (softmax(and_kernel)
-> gamma, the per-(r,d) beta products, sigmoid(or_kernel), diagonal mask)
are precomputed on the host in fp64 and DMAed in pre-broadcast, so the
device spends zero time on the weight path and the binary eval starts as
soon as its DMAs land.

Engine split: the heavy bf16 eval + product trees run on the vector engine
(2x mode); the +1 bias passes run on the scalar engine; gpsimd is used for
bulk/late input DMAs. The final per-object reductions use one PE transpose
of the [128, 96] disjunct-complement tile.
"""

import numpy as np
import ml_dtypes

BF = ml_dtypes.bfloat16
B, N, P0, P1, P2, R, D = 32, 32, 16, 32, 16, 3, 8
RD = R * D              # 24
NCORE = 8
BL = B // NCORE         # 4 batches per core
NT = BL * 8             # 32 row-tiles of 128 per core

_CACHE = {}


def _build():
    import concourse.tile as tile
    from concourse import mybir, bacc

    F32 = mybir.dt.float32
    B16 = mybir.dt.bfloat16
    MUL = mybir.AluOpType.mult
    ADD = mybir.AluOpType.add
    AF = mybir.ActivationFunctionType

    nc = bacc.Bacc("TRN2", target_bir_lowering=False, debug=False,
                   num_devices=NCORE)

    # ---- parameters (per-core shards / replicated constants) ----
    x_all_in = nc.declare_dram_parameter("x_all", [128, NT * 32], B16, isOutput=False)
    xu_in = nc.declare_dram_parameter("xu", [128, 80], B16, isOutput=False)
    gB_in = nc.declare_dram_parameter("gB", [1, 768], B16, isOutput=False)
    gU_in = nc.declare_dram_parameter("gU", [1, 1536], B16, isOutput=False)
    gN_in = nc.declare_dram_parameter("gN", [128, 384], B16, isOutput=False)
    okm_in = nc.declare_dram_parameter("okm", [128, 192], B16, isOutput=False)
    sel_in = nc.declare_dram_parameter("selcat", [32, 1152], B16, isOutput=False)
    ident_in = nc.declare_dram_parameter("ident", [128, 128], F32, isOutput=False)
    oldb_in = nc.declare_dram_parameter("olds_bin", [128, NT], F32, isOutput=False)
    oldu_in = nc.declare_dram_parameter("olds_un", [33, 4], F32, isOutput=False)

    out_binm = nc.declare_dram_parameter("out_binm", [128, NT], F32, isOutput=True)
    out_unm = nc.declare_dram_parameter("out_unm", [33, 4], F32, isOutput=True)

    with tile.TileContext(nc) as tc:
        with tc.tile_pool(name="cb", bufs=1) as cb, \
             tc.tile_pool(name="wk", bufs=1) as wk, \
             tc.tile_pool(name="ps", bufs=1, space="PSUM") as ps:

            # ---------- t0: input DMAs ----------
            # sync queue carries only the latency-critical chain (each
            # dma_start costs ~600ns of SP sequencer time, serialized);
            # bulk/late tensors ride the gpsimd DGE.
            gB = cb.tile([128, 768], B16)
            nc.sync.dma_start(gB[:], gB_in[:].broadcast_to((128, 768)))
            x_all = cb.tile([128, NT * 32], B16)
            for h in range(4):
                nc.sync.dma_start(x_all[:, h * 256:(h + 1) * 256],
                                  x_all_in[:, h * 256:(h + 1) * 256])
            xu = cb.tile([128, 80], B16)
            nc.gpsimd.dma_start(xu[:], xu_in[:])
            gUs = cb.tile([128, 1536], B16)
            nc.gpsimd.dma_start(gUs[:], gU_in[:].broadcast_to((128, 1536)))
            gNs = cb.tile([128, 384], B16)
            nc.gpsimd.dma_start(gNs[:], gN_in[:])
            okmB = cb.tile([128, 192], B16)
            nc.gpsimd.dma_start(okmB[:], okm_in[:])
            sel = cb.tile([32, 1152], B16)
            nc.gpsimd.dma_start(sel[:], sel_in[:])
            ident = cb.tile([128, 128], F32)
            nc.gpsimd.dma_start(ident[:], ident_in[:])
            oldb = cb.tile([128, NT], F32)
            nc.gpsimd.dma_start(oldb[:], oldb_in[:])
            oldu = cb.tile([33, 4], F32)
            nc.gpsimd.dma_start(oldu[:], oldu_in[:])

            # ---------- phase C: unary/nullary factor pass ----
            emU = wk.tile([128, 1536], B16)

            def _emu(dst_lo, n_grp, x_lo, src_lo):
                nc.vector.tensor_tensor(
                    emU[:, dst_lo:dst_lo + n_grp * 32]
                        .rearrange("p (g c) -> p g c", c=32),
                    xu[:, x_lo:x_lo + 32].unsqueeze(1)
                        .broadcast_to((128, n_grp, 32)),
                    gUs[:, src_lo:src_lo + n_grp * 32]
                        .rearrange("p (g c) -> p g c", c=32), op=MUL)
            _emu(0, 16, 0, 0)
            _emu(512, 8, 0, 512)
            _emu(768, 8, 32, 768)
            _emu(1024, 16, 32, 1024)
            nc.scalar.activation(emU[:], emU[:], AF.Copy, bias=1.0)
            # U tree: [128, 48, 32] -> [128, 48]
            cur = emU[:].rearrange("p (g c) -> p g c", c=32)
            for w in (16, 8, 4, 2):
                nxt = wk.tile([128, 48 * w], B16, tag=f"ut{w}")
                nc.vector.tensor_tensor(
                    nxt[:].rearrange("p (g c) -> p g c", c=w),
                    cur[:, :, 0:w], cur[:, :, w:2 * w], op=MUL)
                cur = nxt[:].rearrange("p (g c) -> p g c", c=w)
            fu12 = wk.tile([128, 48], B16)
            nc.vector.tensor_tensor(fu12[:].unsqueeze(2), cur[:, :, 0:1],
                                    cur[:, :, 1:2], op=MUL)

            emN = wk.tile([128, 384], B16)
            nc.vector.tensor_tensor(
                emN[:].rearrange("p (r c) -> p r c", r=24),
                xu[:, 64:80].unsqueeze(1).broadcast_to((128, 24, 16)),
                gNs[:].rearrange("p (r c) -> p r c", r=24), op=MUL)
            nc.scalar.activation(emN[:], emN[:], AF.Copy, bias=1.0)
            cur = emN[:].rearrange("p (g c) -> p g c", c=16)
            for w in (8, 4, 2):
                nxt = wk.tile([128, 24 * w], B16, tag=f"nt{w}")
                nc.vector.tensor_tensor(
                    nxt[:].rearrange("p (g c) -> p g c", c=w),
                    cur[:, :, 0:w], cur[:, :, w:2 * w], op=MUL)
                cur = nxt[:].rearrange("p (g c) -> p g c", c=w)
            f0g = wk.tile([128, 24], B16)
            nc.vector.tensor_tensor(f0g[:].unsqueeze(2), cur[:, :, 0:1],
                                    cur[:, :, 1:2], op=MUL)

            fu2f0 = wk.tile([128, 24], B16)
            nc.vector.tensor_tensor(fu2f0[:], fu12[:, 24:48], f0g[:], op=MUL)

            # ---------- phase D: per-b row broadcasts via PE ----------
            FU1B = cb.tile([128, 768], B16)
            FU2F0B = cb.tile([128, 96], B16)
            for b in range(BL):
                rhs1 = wk.tile([32, 24], B16, tag="rhs1")
                nc.vector.tensor_copy(rhs1[:], fu12[b * 32:(b + 1) * 32, 0:24])
                rhs2 = wk.tile([32, 24], B16, tag="rhs2")
                nc.vector.tensor_copy(rhs2[:], fu2f0[b * 32:(b + 1) * 32, :])
                psF = ps.tile([128, 192], F32, tag="pp")
                for t in range(8):
                    nc.tensor.matmul(psF[:, t * 24:(t + 1) * 24],
                                     sel[0:32, t * 128:(t + 1) * 128],
                                     rhs1[:], start=True, stop=True)
                nc.vector.tensor_copy(FU1B[:, b * 192:(b + 1) * 192], psF[:])
                psJ = ps.tile([128, 24], F32, tag="pj")
                nc.tensor.matmul(psJ[:], sel[0:32, 1024:1152],
                                 rhs2[:], start=True, stop=True)
                nc.vector.tensor_copy(FU2F0B[:, b * 24:(b + 1) * 24], psJ[:])

            # PFOK[p, (b,t,rd)] = FU1B * FU2F0B(bcast t) * okmB(bcast b)
            PFOK = cb.tile([128, 768], B16)
            nc.vector.tensor_tensor(
                PFOK[:].rearrange("p (b t r) -> p b t r", b=4, t=8),
                FU1B[:].rearrange("p (b t r) -> p b t r", b=4, t=8),
                FU2F0B[:].rearrange("p (b r) -> p b r", b=4)
                    .unsqueeze(2).broadcast_to((128, 4, 8, 24)), op=MUL)
            nc.vector.tensor_tensor(
                PFOK[:].rearrange("p (b t r) -> p b t r", b=4, t=8),
                PFOK[:].rearrange("p (b t r) -> p b t r", b=4, t=8),
                okmB[:].rearrange("p (t r) -> p t r", t=8)
                    .unsqueeze(1).broadcast_to((128, 4, 8, 24)), op=MUL)

            # ---------- phase E: main binary pipeline ----------
            em = wk.tile([128, NT * 768], B16)
            t1 = wk.tile([128, NT * 384], B16)
            t2 = wk.tile([128, NT * 192], B16)
            t3 = wk.tile([128, NT * 96], B16)
            t4 = wk.tile([128, NT * 48], B16)
            cj = wk.tile([128, NT * 24], B16)
            gA = wk.tile([128, 768], B16)
            d1 = wk.tile([128, 384], B16)
            d2 = wk.tile([128, 192], B16)
            pdF = wk.tile([128, 96], F32)

            # evals (vector 2x) + bias (+1): b0-b2 scalar, b3 vector 4x
            for b in range(BL):
                nc.vector.tensor_tensor(
                    em[:, b * 6144:(b + 1) * 6144]
                        .rearrange("p (k r c) -> p k r c", k=8, r=24),
                    x_all[:, b * 256:(b + 1) * 256]
                        .rearrange("p (k c) -> p k c", k=8)
                        .unsqueeze(2).broadcast_to((128, 8, 24, 32)),
                    gB[:].rearrange("p (r c) -> p r c", r=24)
                        .unsqueeze(1).broadcast_to((128, 8, 24, 32)), op=MUL)
                if b == BL - 1:
                    nc.vector.tensor_scalar(em[:, b * 6144:(b + 1) * 6144],
                                            em[:, b * 6144:(b + 1) * 6144],
                                            1.0, None, op0=ADD)
                else:
                    nc.scalar.activation(em[:, b * 6144:(b + 1) * 6144],
                                         em[:, b * 6144:(b + 1) * 6144],
                                         AF.Copy, bias=1.0)

            # L1/L2 per b (pipelines behind each +1), L3..pd per b-pair
            for b in range(BL):
                cur = em[:, b * 6144:(b + 1) * 6144].rearrange(
                    "p (g c) -> p g c", c=32)
                for w, tl in ((16, t1), (8, t2)):
                    dst = tl[:, b * 192 * w:(b + 1) * 192 * w].rearrange(
                        "p (g c) -> p g c", c=w)
                    nc.vector.tensor_tensor(dst, cur[:, :, 0:w],
                                            cur[:, :, w:2 * w], op=MUL)
                    cur = dst
            for pb in range(2):
                cur = t2[:, pb * 3072:(pb + 1) * 3072].rearrange(
                    "p (g c) -> p g c", c=8)
                for w, tl in ((4, t3), (2, t4)):
                    dst = tl[:, pb * 384 * w:(pb + 1) * 384 * w].rearrange(
                        "p (g c) -> p g c", c=w)
                    nc.vector.tensor_tensor(dst, cur[:, :, 0:w],
                                            cur[:, :, w:2 * w], op=MUL)
                    cur = dst
                # disjunct chain per pair
                nc.vector.tensor_tensor(
                    cj[:, pb * 384:(pb + 1) * 384].unsqueeze(2),
                    cur[:, :, 0:1], cur[:, :, 1:2], op=MUL)
                cjb = cj[:, pb * 384:(pb + 1) * 384]
                nc.vector.tensor_tensor(cjb, cjb,
                                        PFOK[:, pb * 384:(pb + 1) * 384],
                                        op=MUL)
                gAb = gA[:, pb * 384:(pb + 1) * 384]
                nc.vector.tensor_scalar(gAb, cjb, -1.0, 1.0, op0=MUL, op1=ADD)
                d1b = d1[:, pb * 192:(pb + 1) * 192].rearrange(
                    "p (g dd) -> p g dd", dd=4)
                gvb = gAb.rearrange("p (g dd) -> p g dd", dd=8)
                nc.vector.tensor_tensor(d1b, gvb[:, :, 0:4], gvb[:, :, 4:8],
                                        op=MUL)
                d2b = d2[:, pb * 96:(pb + 1) * 96].rearrange(
                    "p (g dd) -> p g dd", dd=2)
                nc.vector.tensor_tensor(d2b, d1b[:, :, 0:2], d1b[:, :, 2:4],
                                        op=MUL)
                # write (r, k)-ordered pd: pdF[p, r*32 + b*8 + t]  (fp32 out)
                d2b4 = d2[:, pb * 96:(pb + 1) * 96].rearrange(
                    "p (b t r dd) -> p b t r dd", b=2, t=8, r=3)
                nc.vector.tensor_tensor(
                    pdF[:].rearrange("p (r k) -> p r k", r=3)
                        [:, :, pb * 16:(pb + 1) * 16]
                        .rearrange("p r (b t) -> p b t r", b=2).unsqueeze(4),
                    d2b4[:, :, :, :, 0:1], d2b4[:, :, :, :, 1:2], op=MUL)

                # binary merge per pair (r=2 block cols), early output DMA
                tb = wk.tile([128, 16], F32, tag=f"tb{pb}")
                ob = oldb[:, pb * 16:(pb + 1) * 16]
                nc.vector.tensor_scalar(tb[:], ob, -1.0, 1.0,
                                        op0=MUL, op1=ADD)
                nc.vector.tensor_tensor(
                    tb[:], tb[:], pdF[:, 64 + pb * 16:64 + (pb + 1) * 16],
                    op=MUL)
                nc.vector.tensor_scalar(tb[:], tb[:], -1.0, 1.0,
                                        op0=MUL, op1=ADD)
                nc.gpsimd.dma_start(
                    out_binm[:, pb * 16:(pb + 1) * 16], tb[:])

            # ---------- phase F: unary/nullary merges via PE transpose ----------
            pdT = ps.tile([96, 128], F32, tag="pt")
            nc.tensor.transpose(pdT[:], pdF[:], ident[:])
            pdS = wk.tile([64, 128], F32)
            nc.vector.tensor_copy(pdS[:], pdT[0:64, :])

            # shared product over j within i4-groups for rows 0:64 (r=0, r=1)
            cur = pdS[:].rearrange("p (i4 j) -> p i4 j", i4=4)
            for w in (16, 8, 4, 2, 1):
                nxt = wk.tile([64, 4 * w], F32, tag=f"pu{w}")
                nxtv = nxt[:].rearrange("p (i4 j) -> p i4 j", i4=4)
                nc.vector.tensor_tensor(nxtv, cur[:, :, 0:w], cur[:, :, w:2 * w],
                                        op=MUL)
                cur = nxtv
            j4 = nxt  # [64, 4]: rows 0:32 = r0 per (b,t,i4); rows 32:64 = r1
            pdu0 = wk.tile([32, 4], F32)
            nc.vector.tensor_copy(pdu0[:], j4[:][32:64, :])
            pdu = pdu0[:]
            tu = wk.tile([33, 4], F32)
            nc.vector.tensor_scalar(tu[0:32, :], oldu[0:32, :], -1.0, 1.0,
                                    op0=MUL, op1=ADD)
            nc.vector.tensor_tensor(tu[0:32, :], tu[0:32, :], pdu, op=MUL)

            # nullary: finish the product over i4 for rows 0:32 (r=0)
            cur = j4[:][0:32, :]
            for w in (2, 1):
                nxt = wk.tile([32, w], F32, tag=f"pn{w}")
                nc.vector.tensor_tensor(nxt[:], cur[:, 0:w], cur[:, w:2 * w],
                                        op=MUL)
                cur = nxt[:]
            # fold the remaining 32 partition values (b, t) -> per-b products
            q = wk.tile([32, 32], F32)
            nc.vector.memset(q[:], 1.0)
            nc.vector.tensor_copy(q[:, 0:1], cur)
            qT = wk.tile([32, 32], F32)
            nc.vector.transpose(qT[:], q[:])
            cur = qT[0:1, :].rearrange("p (b i8) -> p b i8", b=4)
            for w in (4, 2, 1):
                nxt = wk.tile([1, 4 * w], F32, tag=f"pq{w}")
                nxtv = nxt[:].rearrange("p (b i8) -> p b i8", b=4)
                nc.vector.tensor_tensor(nxtv, cur[:, :, 0:w], cur[:, :, w:2 * w],
                                        op=MUL)
                cur = nxtv
            pdn = cur.rearrange("p b i8 -> p (b i8)")  # [1, 4]
            on0 = wk.tile([1, 4], F32)
            nc.vector.tensor_copy(on0[:], oldu[32:33, :])
            tn = wk.tile([1, 4], F32)
            nc.vector.tensor_scalar(tn[:], on0[:], -1.0, 1.0, op0=MUL, op1=ADD)
            nc.vector.tensor_tensor(tn[:], tn[:], pdn, op=MUL)
            nc.vector.tensor_copy(tu[32:33, :], tn[:])
            nc.vector.tensor_scalar(tu[:], tu[:], -1.0, 1.0, op0=MUL, op1=ADD)
            nc.sync.dma_start(out_unm[:], tu[:])

    nc.compile()
    return nc


def _host_prep(nullary_preds, unary_preds, binary_preds, and_kernel, or_kernel):
    """Build per-core input maps (sharding + layout + weight-only prep)."""
    null_ = np.asarray(nullary_preds, np.float32)
    un = np.asarray(unary_preds, np.float32)
    bi = np.asarray(binary_preds, np.float32)
    ak = np.asarray(and_kernel, np.float64)
    ok = np.asarray(or_kernel, np.float64)

    I, J = np.meshgrid(np.arange(N), np.arange(N), indexing="ij")
    off = I != J
    Jm = J - (J > I)
    Im = I - (I > J)

    binP = np.zeros((B, N, N, P2), np.float32)
    binP[:, off] = bi[:, I[off], Jm[off]]
    binT = np.zeros((B, N, N, P2), np.float32)
    binT[:, off] = bi[:, J[off], Im[off]]
    binPT = np.concatenate([binP, binT], axis=-1)          # [B,32,32,32]

    # row-tile layout: x_all[core][p, k=(b,t), c] = binPT[4c+b, t*128+p, c]
    xg = binPT.reshape(NCORE, BL, 8, 128, 32)
    x_all = np.ascontiguousarray(xg.transpose(0, 3, 1, 2, 4)
                                 ).reshape(NCORE, 128, NT * 32).astype(BF)
    olds_bin = np.ascontiguousarray(
        binP[..., 15].reshape(NCORE, BL, 8, 128).transpose(0, 3, 1, 2)
    ).reshape(NCORE, 128, NT).astype(np.float32)

    # unary pass rows (b, i): [u | u | n]
    xun = np.concatenate(
        [un, un, np.broadcast_to(null_[:, None, :], (B, N, P0))], axis=-1)
    xu = xun.reshape(NCORE, 128, 80).astype(BF)
    # rows (b, i8), cols i4 : out_unm[q=(b*8+i//4), i%4]; row 32 = nullary
    olds_un = np.concatenate(
        [un[..., 31].reshape(NCORE, 4, 8, 4).reshape(NCORE, 32, 4),
         null_[:, 15].reshape(NCORE, 1, 4)], axis=1).astype(np.float32)

    # weight-only prep (fp64): softmax(and_kernel) -> gamma form + OR fold
    akT = np.ascontiguousarray(ak.transpose(2, 0, 1, 3)).reshape(112, RD, 3)
    e = np.exp(akT - akT.max(axis=-1, keepdims=True))      # [c, rd, 3]
    bsum = e[:, :, 1] + e[:, :, 2]
    stot = e.sum(axis=-1)
    gam = (e[:, :, 0] - e[:, :, 1]) / bsum                 # [c, rd]
    bprod = (bsum / stot).prod(axis=0)                     # [rd]
    sig = 1.0 / (1.0 + np.exp(-ok.reshape(RD)))            # [rd]
    okrow = sig * bprod                                    # [rd]

    gamT = gam.T                                           # [rd, c]
    gB = gamT[:, 80:112].reshape(1, 768).astype(BF)
    gU = gamT[:, 16:80].reshape(1, 1536).astype(BF)
    gN = np.broadcast_to(gamT[:, 0:16].reshape(1, 384), (128, 384)).astype(BF)

    p = np.arange(128)
    t = np.arange(8)
    maskc = ((p[:, None] % 32) != (t[None, :] * 4 + p[:, None] // 32)
             ).astype(np.float64)
    okm = (maskc[:, :, None] * okrow[None, None, :]).reshape(128, 192).astype(BF)

    selT = (np.arange(32)[:, None, None] == (t[None, :, None] * 4 + p[None, None, :] // 32))
    selJ = (np.arange(32)[:, None] == (p[None, :] % 32))
    selcat = np.concatenate([selT.reshape(32, 1024), selJ], axis=1).astype(BF)
    ident = np.eye(128, dtype=np.float32)

    in_maps = []
    for c in range(NCORE):
        in_maps.append({
            "x_all": x_all[c],
            "xu": xu[c],
            "gB": gB,
            "gU": gU,
            "gN": gN,
            "okm": okm,
            "selcat": selcat,
            "ident": ident,
            "olds_bin": olds_bin[c],
            "olds_un": olds_un[c],
        })
    return in_maps


def _assemble(results, nullary_preds, unary_preds, binary_preds):
    null_ = np.asarray(nullary_preds, np.float32).copy()
    un = np.asarray(unary_preds, np.float32).copy()
    bi = np.asarray(binary_preds, np.float32).copy()

    I, J = np.meshgrid(np.arange(N), np.arange(N), indexing="ij")
    off = I != J
    Jm = J - (J > I)

    for c in range(NCORE):
        r = results[c]
        # out_binm [128, NT=(b,t)] -> rows[b, t*128+p]
        ob = r["out_binm"].reshape(128, BL, 8).transpose(1, 2, 0).reshape(BL, N, N)
        for bl in range(BL):
            b = c * BL + bl
            bi[b, I[off], Jm[off], 15] = ob[bl][off]
        un[c * BL:(c + 1) * BL, :, 31] = r["out_unm"][0:32].reshape(BL, 8, 4).reshape(BL, N)
        null_[c * BL:(c + 1) * BL, 15] = r["out_unm"][32].reshape(BL)

    return np.concatenate(
        [null_, un.reshape(B, -1), bi.reshape(B, -1)], axis=-1)


def kernel(nullary_preds, unary_preds, binary_preds, and_kernel, or_kernel):
    from concourse.bass_utils import run_bass_kernel_spmd

    if "nc" not in _CACHE:
        _CACHE["nc"] = _build()
    nc = _CACHE["nc"]

    in_maps = _host_prep(nullary_preds, unary_preds, binary_preds,
                         and_kernel, or_kernel)
    res = run_bass_kernel_spmd(nc, in_maps, list(range(NCORE)))
    return _assemble(res.results, nullary_preds, unary_preds, binary_preds)


if __name__ == "__main__":
    import reference as ref
    ins = {k: np.asarray(v) for k, v in ref.setup_inputs().items()}
    out = kernel(**ins)
    print("kernel out:", out.shape, out.dtype)
